# revision 30
# baseline (speedup 1.0000x reference)
"""DifferentiableTokenSelection Trainium2 kernel (bf16 pipeline).

Math (reference):
    x: [b=2, t=64, n=1024, e=512] -> x_flat [b, m=65536, e]
    scores  = x_flat @ W.T + bias            [b, m, k=256]
    weights = softmax(scores / tau, axis=m)  (tau = 1.0)
    out     = einsum('bmk,bme->bke', weights, x_flat)   [b, 256, 512]

Key simplifications (exact, not approximations):
  * softmax over m is invariant to per-(b,k) constant shifts -> the bias
    cancels entirely; ignore b_bias.
  * scores ~ N(0,1), max |s| ~ 6 -> exp() without max-subtraction is safe
    in fp32. Single streaming pass: U[k,e] = sum_m exp(s[m,k]) x[m,e] and
    denom[k] = sum_m exp(s[m,k]) accumulate in PSUM; out = U / denom.
  * numerator and denominator use the SAME quantized weights, so weight
    quantization largely cancels in the ratio.

Inputs are cast to bf16 on the host. This halves HBM traffic (x read
twice: once natural for the pooling matmul, once via the hardware DMA
transpose engine for the scores matmul) and runs all matmuls at the PE's
full 1 cycle/row rate with hidden (FWL) weight loads — fp32 matmuls are
4x slower and fp32r fused loads serialize. PSUM accumulation stays fp32.

Sharding: batch x token-axis. core i handles batch i//4, m-rows
[16384*(i%4), 16384*(i%4+1)). Each core emits partial U and denom; the
host sums the 4 partials per batch and divides (gather/unshard step).

Per-core pipeline over 512-row blocks (4x 128-row subtiles):
  DMA x natural [128,4,512] + 4x DMA-transpose xT chunks [128e, 512m];
  per subtile: mm1 scores[m,k] (4 K-chunks, PSUM accum) -> ACT exp ->
  mm2 U[k,e] += wexp^T @ x and den[k] += wexp^T @ ones (PSUM, full-kernel
  accumulation).
"""

import numpy as np
import ml_dtypes

import concourse.bacc as bacc
import concourse.bass as bass
import concourse.tile as tile
from concourse import mybir
from concourse.bass_utils import run_bass_kernel_spmd

B, T, NTOK, E, K = 2, 64, 1024, 512, 256
M = T * NTOK                 # 65536 tokens per batch
NCORES = 8
CORES_PER_B = NCORES // B    # 4
RPC = M // CORES_PER_B       # 16384 rows per core

F32 = mybir.dt.float32
BF16 = mybir.dt.bfloat16
EXP = mybir.ActivationFunctionType.Exp
BF = ml_dtypes.bfloat16

# const layout per partition: [ ones(2) | wt(4*256) ]
C_ONES, C_WT = 0, 2
C_TOT = 2 + 4 * K


def build_nc(rows: int, subs_per_blk: int = 8) -> bass.Bass:
    """Emit the per-core bass program for `rows` m-rows."""
    assert rows % (128 * subs_per_blk) == 0
    nsub = rows // 128
    nblk = nsub // subs_per_blk

    nc = bacc.Bacc("TRN2", target_bir_lowering=False, debug=False)
    x_d = nc.dram_tensor("x", [rows, E], BF16, kind="ExternalInput")
    # second copy of x in e-chunk-major layout: xc[ec, m, :] = x[m, 128ec:..]
    # -> each DMA-transpose reads one fully contiguous slab (fast xbar path)
    xc_d = nc.dram_tensor("xc", [4, rows, 128], BF16, kind="ExternalInput")
    c_d = nc.dram_tensor("consts", [128, C_TOT], BF16, kind="ExternalInput")
    u_d = nc.dram_tensor("u", [2, 128, E], F32, kind="ExternalOutput")
    # per-partition-lane partial denominator; host sums over partitions
    d_d = nc.dram_tensor("dacc", [128, K], F32, kind="ExternalOutput")

    with tile.TileContext(nc) as tc:
        with (
            tc.tile_pool(name="const", bufs=1) as constp,
            tc.tile_pool(name="xin", bufs=6) as xinp,
            tc.tile_pool(name="xt", bufs=6) as xtp,
            tc.tile_pool(name="wexp", bufs=3) as wexpp,
            tc.tile_pool(name="outs", bufs=1) as outp,
            tc.tile_pool(name="ps_sc", bufs=3, space="PSUM") as ps_sc,
            tc.tile_pool(name="ps_acc", bufs=1, space="PSUM") as ps_acc,
        ):
            consts = constp.tile([128, C_TOT], BF16)
            nc.sync.dma_start(out=consts[:], in_=c_d.ap())

            u_ps = ps_acc.tile([128, 2, E], F32)    # 2 banks, live all kernel
            dacc = outp.tile([128, K], F32)         # denominator accumulator
            nc.gpsimd.memset(dacc[:], 0.0)

            for blk in range(nblk):
                r0 = blk * subs_per_blk * 128
                xb = xinp.tile([128, subs_per_blk, E], BF16, tag="xb")
                # natural loads ride SWDGE (gpsimd) so the HWDGE (sync)
                # stream carries ONLY xbar-transposes -> no xbar-mode
                # transition serialization between the two DMA kinds
                nc.gpsimd.dma_start(
                    out=xb[:],
                    in_=x_d.ap()[r0 : r0 + subs_per_blk * 128, :].rearrange(
                        "(j p) e -> p j e", p=128
                    ),
                )
                # xT chunks via the DMA transpose engine:
                # xtb[:, ec, :] = x[r0:r0+512, 128ec:128ec+128].T
                xtb = xtp.tile([128, 4, subs_per_blk * 128], BF16, tag="xtb")
                for ec in range(4):
                    # split across both HWDGE engines (sync + scalar)
                    eng = nc.sync if ec % 2 == 0 else nc.scalar
                    eng.dma_start_transpose(
                        xtb[:, ec, :],
                        xc_d.ap()[ec, r0 : r0 + subs_per_blk * 128, :],
                    )
                for j in range(subs_per_blk):
                    it = blk * subs_per_blk + j
                    first, last = it == 0, it == nsub - 1
                    # -- mm1: scores[m,k] = sum_e x[m,e] WT[e,k]
                    sc_ps = ps_sc.tile([128, K], F32, tag="scps")
                    for ec in range(4):
                        nc.tensor.matmul(
                            sc_ps[:],
                            xtb[:, ec, j * 128 : (j + 1) * 128],
                            consts[:, C_WT + ec * K : C_WT + (ec + 1) * K],
                            start=(ec == 0),
                            stop=(ec == 3),
                        )
                    # -- exp (tau=1, bias cancels)
                    wexp = wexpp.tile([128, K], BF16, tag="wexp")
                    nc.scalar.activation(wexp[:], sc_ps[:], EXP)
                    # -- mm2: U[k,e] += wexp^T @ x
                    for c in range(2):
                        nc.tensor.matmul(
                            u_ps[:, c, :],
                            wexp[:, c * 128 : (c + 1) * 128],
                            xb[:, j, :],
                            start=first,
                            stop=last,
                        )
                    # -- denominator: per-lane running sum on DVE (host does
                    # the final partition reduction)
                    nc.vector.tensor_add(dacc[:], dacc[:], wexp[:])

            u_sb = outp.tile([128, 2, E], F32)
            nc.vector.tensor_copy(u_sb[:], u_ps[:])
            nc.sync.dma_start(
                out=u_d.ap().rearrange("c p e -> p c e"), in_=u_sb[:]
            )
            nc.sync.dma_start(out=d_d.ap(), in_=dacc[:])
    nc.compile()
    return nc


def _run(nc: bass.Bass, in_maps, **kw):
    return run_bass_kernel_spmd(nc, in_maps, list(range(len(in_maps))), **kw)


def make_consts(W: np.ndarray) -> np.ndarray:
    """[ones | W.T as [c p] k chunks] per partition, bf16."""
    consts = np.zeros((128, C_TOT), BF)
    consts[:, C_ONES : C_ONES + 2] = BF(1.0)
    wt = np.ascontiguousarray(W.T, np.float32).astype(BF)  # [E, K]
    for c in range(4):
        consts[:, C_WT + c * K : C_WT + (c + 1) * K] = wt[
            c * 128 : (c + 1) * 128, :
        ]
    return consts


def make_in_maps(x: np.ndarray, W: np.ndarray):
    xf = np.asarray(x, np.float32).reshape(B, M, E).astype(BF)
    consts = make_consts(W)
    in_maps = []
    for i in range(NCORES):
        bi, si = divmod(i, CORES_PER_B)
        shard = np.ascontiguousarray(xf[bi, si * RPC : (si + 1) * RPC])
        chunked = np.ascontiguousarray(
            shard.reshape(RPC, 4, 128).transpose(1, 0, 2)
        )
        in_maps.append({"x": shard, "xc": chunked, "consts": consts})
    return in_maps


def combine(results) -> np.ndarray:
    """Sum per-core partials per batch, normalize, stack."""
    out = np.empty((B, K, E), np.float32)
    for bi in range(B):
        U = np.zeros((K, E), np.float64)
        den = np.zeros((K,), np.float64)
        for si in range(CORES_PER_B):
            r = results[bi * CORES_PER_B + si]
            U += r["u"].reshape(K, E).astype(np.float64)
            den += r["dacc"].astype(np.float64).sum(axis=0)
        out[bi] = (U / den[:, None]).astype(np.float32)
    return out


_NC_CACHE: dict[int, bass.Bass] = {}


def kernel(x: np.ndarray, W: np.ndarray, b_bias: np.ndarray) -> np.ndarray:
    # b_bias shifts every column of scores by a constant along the softmax
    # axis -> cancels in softmax; unused by construction.
    if RPC not in _NC_CACHE:
        _NC_CACHE[RPC] = build_nc(RPC)
    res = _run(_NC_CACHE[RPC], make_in_maps(np.asarray(x), np.asarray(W)))
    return combine(res.results)


# revision 31
# speedup vs baseline: 1.0777x; 1.0777x over previous
"""DifferentiableTokenSelection Trainium2 kernel (bf16 pipeline).

Math (reference):
    x: [b=2, t=64, n=1024, e=512] -> x_flat [b, m=65536, e]
    scores  = x_flat @ W.T + bias            [b, m, k=256]
    weights = softmax(scores / tau, axis=m)  (tau = 1.0)
    out     = einsum('bmk,bme->bke', weights, x_flat)   [b, 256, 512]

Key simplifications (exact, not approximations):
  * softmax over m is invariant to per-(b,k) constant shifts -> the bias
    cancels entirely; ignore b_bias.
  * scores ~ N(0,1), max |s| ~ 6 -> exp() without max-subtraction is safe
    in fp32. Single streaming pass: U[k,e] = sum_m exp(s[m,k]) x[m,e] and
    denom[k] = sum_m exp(s[m,k]) accumulate in PSUM; out = U / denom.
  * numerator and denominator use the SAME quantized weights, so weight
    quantization largely cancels in the ratio.

Inputs are cast to bf16 on the host. This halves HBM traffic (x read
twice: once natural for the pooling matmul, once via the hardware DMA
transpose engine for the scores matmul) and runs all matmuls at the PE's
full 1 cycle/row rate with hidden (FWL) weight loads — fp32 matmuls are
4x slower and fp32r fused loads serialize. PSUM accumulation stays fp32.

Sharding: batch x token-axis. core i handles batch i//4, m-rows
[16384*(i%4), 16384*(i%4+1)). Each core emits partial U and denom; the
host sums the 4 partials per batch and divides (gather/unshard step).

Per-core pipeline over 512-row blocks (4x 128-row subtiles):
  DMA x natural [128,4,512] + 4x DMA-transpose xT chunks [128e, 512m];
  per subtile: mm1 scores[m,k] (4 K-chunks, PSUM accum) -> ACT exp ->
  mm2 U[k,e] += wexp^T @ x and den[k] += wexp^T @ ones (PSUM, full-kernel
  accumulation).
"""

import numpy as np
import ml_dtypes

import concourse.bacc as bacc
import concourse.bass as bass
import concourse.tile as tile
from concourse import mybir
from concourse.bass_utils import run_bass_kernel_spmd

B, T, NTOK, E, K = 2, 64, 1024, 512, 256
M = T * NTOK                 # 65536 tokens per batch
NCORES = 8
CORES_PER_B = NCORES // B    # 4
RPC = M // CORES_PER_B       # 16384 rows per core

F32 = mybir.dt.float32
BF16 = mybir.dt.bfloat16
EXP = mybir.ActivationFunctionType.Exp
BF = ml_dtypes.bfloat16

# const layout per partition: [ ones(2) | wt(4*256) ]
C_ONES, C_WT = 0, 2
C_TOT = 2 + 4 * K


def build_nc(rows: int, subs_per_blk: int = 8) -> bass.Bass:
    """Emit the per-core bass program for `rows` m-rows."""
    assert rows % (128 * subs_per_blk) == 0
    nsub = rows // 128
    nblk = nsub // subs_per_blk

    nc = bacc.Bacc("TRN2", target_bir_lowering=False, debug=False)
    x_d = nc.dram_tensor("x", [rows, E], BF16, kind="ExternalInput")
    # second copy of x in e-chunk-major layout: xc[ec, m, :] = x[m, 128ec:..]
    # -> each DMA-transpose reads one fully contiguous slab (fast xbar path)
    xc_d = nc.dram_tensor("xc", [4, rows, 128], BF16, kind="ExternalInput")
    c_d = nc.dram_tensor("consts", [128, C_TOT], BF16, kind="ExternalInput")
    u_d = nc.dram_tensor("u", [2, 128, E], F32, kind="ExternalOutput")
    # per-partition-lane partial denominator; host sums over partitions
    d_d = nc.dram_tensor("dacc", [128, K], F32, kind="ExternalOutput")

    with tile.TileContext(nc) as tc:
        with (
            tc.tile_pool(name="const", bufs=1) as constp,
            tc.tile_pool(name="xin", bufs=6) as xinp,
            tc.tile_pool(name="xt", bufs=6) as xtp,
            tc.tile_pool(name="wexp", bufs=3) as wexpp,
            tc.tile_pool(name="outs", bufs=1) as outp,
            tc.tile_pool(name="ps_sc", bufs=3, space="PSUM") as ps_sc,
            tc.tile_pool(name="ps_acc", bufs=1, space="PSUM") as ps_acc,
        ):
            consts = constp.tile([128, C_TOT], BF16)
            nc.sync.dma_start(out=consts[:], in_=c_d.ap())

            u_ps = ps_acc.tile([128, 2, E], F32)    # 2 banks, live all kernel
            dacc = outp.tile([128, K], F32)         # denominator accumulator
            nc.gpsimd.memset(dacc[:], 0.0)

            for blk in range(nblk):
                r0 = blk * subs_per_blk * 128
                xb = xinp.tile([128, subs_per_blk, E], BF16, tag="xb")
                # natural loads ride SWDGE (gpsimd) so the HWDGE (sync)
                # stream carries ONLY xbar-transposes -> no xbar-mode
                # transition serialization between the two DMA kinds
                nc.gpsimd.dma_start(
                    out=xb[:],
                    in_=x_d.ap()[r0 : r0 + subs_per_blk * 128, :].rearrange(
                        "(j p) e -> p j e", p=128
                    ),
                )
                # xT chunks via the DMA transpose engine:
                # xtb[:, ec, :] = x[r0:r0+512, 128ec:128ec+128].T
                xtb = xtp.tile([128, 4, subs_per_blk * 128], BF16, tag="xtb")
                for ec in range(4):
                    nc.sync.dma_start_transpose(
                        xtb[:, ec, :],
                        xc_d.ap()[ec, r0 : r0 + subs_per_blk * 128, :],
                    )
                for j in range(subs_per_blk):
                    it = blk * subs_per_blk + j
                    first, last = it == 0, it == nsub - 1
                    # -- mm1: scores[m,k] = sum_e x[m,e] WT[e,k]
                    sc_ps = ps_sc.tile([128, K], F32, tag="scps")
                    for ec in range(4):
                        nc.tensor.matmul(
                            sc_ps[:],
                            xtb[:, ec, j * 128 : (j + 1) * 128],
                            consts[:, C_WT + ec * K : C_WT + (ec + 1) * K],
                            start=(ec == 0),
                            stop=(ec == 3),
                        )
                    # -- exp (tau=1, bias cancels)
                    wexp = wexpp.tile([128, K], BF16, tag="wexp")
                    nc.scalar.activation(wexp[:], sc_ps[:], EXP)
                    # -- mm2: U[k,e] += wexp^T @ x
                    for c in range(2):
                        nc.tensor.matmul(
                            u_ps[:, c, :],
                            wexp[:, c * 128 : (c + 1) * 128],
                            xb[:, j, :],
                            start=first,
                            stop=last,
                        )
                    # -- denominator: per-lane running sum on DVE (host does
                    # the final partition reduction)
                    nc.vector.tensor_add(dacc[:], dacc[:], wexp[:])

            u_sb = outp.tile([128, 2, E], F32)
            nc.vector.tensor_copy(u_sb[:], u_ps[:])
            nc.sync.dma_start(
                out=u_d.ap().rearrange("c p e -> p c e"), in_=u_sb[:]
            )
            nc.sync.dma_start(out=d_d.ap(), in_=dacc[:])
    nc.compile()
    return nc


def _run(nc: bass.Bass, in_maps, **kw):
    return run_bass_kernel_spmd(nc, in_maps, list(range(len(in_maps))), **kw)


def make_consts(W: np.ndarray) -> np.ndarray:
    """[ones | W.T as [c p] k chunks] per partition, bf16."""
    consts = np.zeros((128, C_TOT), BF)
    consts[:, C_ONES : C_ONES + 2] = BF(1.0)
    wt = np.ascontiguousarray(W.T, np.float32).astype(BF)  # [E, K]
    for c in range(4):
        consts[:, C_WT + c * K : C_WT + (c + 1) * K] = wt[
            c * 128 : (c + 1) * 128, :
        ]
    return consts


def make_in_maps(x: np.ndarray, W: np.ndarray):
    xf = np.asarray(x, np.float32).reshape(B, M, E).astype(BF)
    consts = make_consts(W)
    in_maps = []
    for i in range(NCORES):
        bi, si = divmod(i, CORES_PER_B)
        shard = np.ascontiguousarray(xf[bi, si * RPC : (si + 1) * RPC])
        chunked = np.ascontiguousarray(
            shard.reshape(RPC, 4, 128).transpose(1, 0, 2)
        )
        in_maps.append({"x": shard, "xc": chunked, "consts": consts})
    return in_maps


def combine(results) -> np.ndarray:
    """Sum per-core partials per batch, normalize, stack."""
    out = np.empty((B, K, E), np.float32)
    for bi in range(B):
        U = np.zeros((K, E), np.float64)
        den = np.zeros((K,), np.float64)
        for si in range(CORES_PER_B):
            r = results[bi * CORES_PER_B + si]
            U += r["u"].reshape(K, E).astype(np.float64)
            den += r["dacc"].astype(np.float64).sum(axis=0)
        out[bi] = (U / den[:, None]).astype(np.float32)
    return out


_NC_CACHE: dict[int, bass.Bass] = {}


def kernel(x: np.ndarray, W: np.ndarray, b_bias: np.ndarray) -> np.ndarray:
    # b_bias shifts every column of scores by a constant along the softmax
    # axis -> cancels in softmax; unused by construction.
    if RPC not in _NC_CACHE:
        _NC_CACHE[RPC] = build_nc(RPC)
    res = _run(_NC_CACHE[RPC], make_in_maps(np.asarray(x), np.asarray(W)))
    return combine(res.results)


# revision 32
# speedup vs baseline: 1.4326x; 1.3293x over previous
"""DifferentiableTokenSelection Trainium2 kernel (bf16 pipeline).

Math (reference):
    x: [b=2, t=64, n=1024, e=512] -> x_flat [b, m=65536, e]
    scores  = x_flat @ W.T + bias            [b, m, k=256]
    weights = softmax(scores / tau, axis=m)  (tau = 1.0)
    out     = einsum('bmk,bme->bke', weights, x_flat)   [b, 256, 512]

Key simplifications (exact, not approximations):
  * softmax over m is invariant to per-(b,k) constant shifts -> the bias
    cancels entirely; ignore b_bias.
  * scores ~ N(0,1), max |s| ~ 6 -> exp() without max-subtraction is safe
    in fp32. Single streaming pass: U[k,e] = sum_m exp(s[m,k]) x[m,e] and
    denom[k] = sum_m exp(s[m,k]) accumulate in PSUM; out = U / denom.
  * numerator and denominator use the SAME quantized weights, so weight
    quantization largely cancels in the ratio.

Inputs are cast to bf16 on the host. This halves HBM traffic (x read
twice: once natural for the pooling matmul, once via the hardware DMA
transpose engine for the scores matmul) and runs all matmuls at the PE's
full 1 cycle/row rate with hidden (FWL) weight loads — fp32 matmuls are
4x slower and fp32r fused loads serialize. PSUM accumulation stays fp32.

Sharding: batch x token-axis. core i handles batch i//4, m-rows
[16384*(i%4), 16384*(i%4+1)). Each core emits partial U and denom; the
host sums the 4 partials per batch and divides (gather/unshard step).

Per-core pipeline over 512-row blocks (4x 128-row subtiles):
  DMA x natural [128,4,512] + 4x DMA-transpose xT chunks [128e, 512m];
  per subtile: mm1 scores[m,k] (4 K-chunks, PSUM accum) -> ACT exp ->
  mm2 U[k,e] += wexp^T @ x and den[k] += wexp^T @ ones (PSUM, full-kernel
  accumulation).
"""

import numpy as np
import ml_dtypes

import concourse.bacc as bacc
import concourse.bass as bass
import concourse.tile as tile
from concourse import mybir
from concourse.bass_utils import run_bass_kernel_spmd

B, T, NTOK, E, K = 2, 64, 1024, 512, 256
M = T * NTOK                 # 65536 tokens per batch
NCORES = 8
CORES_PER_B = NCORES // B    # 4
RPC = M // CORES_PER_B       # 16384 rows per core

F32 = mybir.dt.float32
BF16 = mybir.dt.bfloat16
EXP = mybir.ActivationFunctionType.Exp
BF = ml_dtypes.bfloat16

# const layout per partition: [ ones(2) | wt(4*256) ]
C_ONES, C_WT = 0, 2
C_TOT = 2 + 4 * K


def build_nc(rows: int, subs_per_blk: int = 16) -> bass.Bass:
    """Emit the per-core bass program for `rows` m-rows."""
    assert rows % (128 * subs_per_blk) == 0
    nsub = rows // 128
    nblk = nsub // subs_per_blk

    nc = bacc.Bacc("TRN2", target_bir_lowering=False, debug=False)
    x_d = nc.dram_tensor("x", [rows, E], BF16, kind="ExternalInput")
    # second copy of x in e-chunk-major layout: xc[ec, m, :] = x[m, 128ec:..]
    # -> each DMA-transpose reads one fully contiguous slab (fast xbar path)
    xc_d = nc.dram_tensor("xc", [4, rows, 128], BF16, kind="ExternalInput")
    c_d = nc.dram_tensor("consts", [128, C_TOT], BF16, kind="ExternalInput")
    u_d = nc.dram_tensor("u", [2, 128, E], F32, kind="ExternalOutput")
    d_d = nc.dram_tensor("d", [128, 2, 2], F32, kind="ExternalOutput")

    with tile.TileContext(nc) as tc:
        with (
            tc.tile_pool(name="const", bufs=1) as constp,
            tc.tile_pool(name="xin", bufs=4) as xinp,
            tc.tile_pool(name="xt", bufs=4) as xtp,
            tc.tile_pool(name="wexp", bufs=3) as wexpp,
            tc.tile_pool(name="outs", bufs=1) as outp,
            tc.tile_pool(name="ps_sc", bufs=3, space="PSUM") as ps_sc,
            tc.tile_pool(name="ps_acc", bufs=1, space="PSUM") as ps_acc,
        ):
            consts = constp.tile([128, C_TOT], BF16)
            nc.sync.dma_start(out=consts[:], in_=c_d.ap())
            ones = consts[:, C_ONES : C_ONES + 2]

            u_ps = ps_acc.tile([128, 2, E], F32)    # 2 banks, live all kernel
            den_ps = ps_acc.tile([128, 2, 2], F32)  # 1 bank; [:, c, :] pairs

            for blk in range(nblk):
                r0 = blk * subs_per_blk * 128
                xb = xinp.tile([128, subs_per_blk, E], BF16, tag="xb")
                # natural loads ride SWDGE (gpsimd) so the HWDGE (sync)
                # stream carries ONLY xbar-transposes -> no xbar-mode
                # transition serialization between the two DMA kinds
                nc.gpsimd.dma_start(
                    out=xb[:],
                    in_=x_d.ap()[r0 : r0 + subs_per_blk * 128, :].rearrange(
                        "(j p) e -> p j e", p=128
                    ),
                )
                # xT chunks via the DMA transpose engine:
                # xtb[:, ec, :] = x[r0:r0+512, 128ec:128ec+128].T
                xtb = xtp.tile([128, 4, subs_per_blk * 128], BF16, tag="xtb")
                for ec in range(4):
                    nc.sync.dma_start_transpose(
                        xtb[:, ec, :],
                        xc_d.ap()[ec, r0 : r0 + subs_per_blk * 128, :],
                    )
                for j in range(subs_per_blk):
                    it = blk * subs_per_blk + j
                    first, last = it == 0, it == nsub - 1
                    # -- mm1: scores[m,k] = sum_e x[m,e] WT[e,k]
                    sc_ps = ps_sc.tile([128, K], F32, tag="scps")
                    for ec in range(4):
                        nc.tensor.matmul(
                            sc_ps[:],
                            xtb[:, ec, j * 128 : (j + 1) * 128],
                            consts[:, C_WT + ec * K : C_WT + (ec + 1) * K],
                            start=(ec == 0),
                            stop=(ec == 3),
                        )
                    # -- exp (tau=1, bias cancels)
                    wexp = wexpp.tile([128, K], BF16, tag="wexp")
                    nc.scalar.activation(wexp[:], sc_ps[:], EXP)
                    # -- mm2: U[k,e] += wexp^T @ x ; den[k] += wexp^T @ 1
                    for c in range(2):
                        wchunk = wexp[:, c * 128 : (c + 1) * 128]
                        nc.tensor.matmul(
                            u_ps[:, c, :],
                            wchunk,
                            xb[:, j, :],
                            start=first,
                            stop=last,
                        )
                        # start=True clears has_written for the WHOLE bank;
                        # both den chunks share one bank, so only chunk 0
                        # issues it (chunk 1 then overwrites where bits are
                        # unset, which is equivalent).
                        nc.tensor.matmul(
                            den_ps[:, c, :],
                            wchunk,
                            ones,
                            start=first and c == 0,
                            stop=last,
                        )

            u_sb = outp.tile([128, 2, E], F32)
            den_sb = outp.tile([128, 2, 2], F32)
            nc.vector.tensor_copy(u_sb[:], u_ps[:])
            nc.vector.tensor_copy(den_sb[:], den_ps[:])
            nc.sync.dma_start(
                out=u_d.ap().rearrange("c p e -> p c e"), in_=u_sb[:]
            )
            nc.sync.dma_start(out=d_d.ap(), in_=den_sb[:])
    nc.compile()
    return nc


def _run(nc: bass.Bass, in_maps, **kw):
    return run_bass_kernel_spmd(nc, in_maps, list(range(len(in_maps))), **kw)


def make_consts(W: np.ndarray) -> np.ndarray:
    """[ones | W.T as [c p] k chunks] per partition, bf16."""
    consts = np.zeros((128, C_TOT), BF)
    consts[:, C_ONES : C_ONES + 2] = BF(1.0)
    wt = np.ascontiguousarray(W.T, np.float32).astype(BF)  # [E, K]
    for c in range(4):
        consts[:, C_WT + c * K : C_WT + (c + 1) * K] = wt[
            c * 128 : (c + 1) * 128, :
        ]
    return consts


def make_in_maps(x: np.ndarray, W: np.ndarray):
    xf = np.asarray(x, np.float32).reshape(B, M, E).astype(BF)
    consts = make_consts(W)
    in_maps = []
    for i in range(NCORES):
        bi, si = divmod(i, CORES_PER_B)
        shard = np.ascontiguousarray(xf[bi, si * RPC : (si + 1) * RPC])
        chunked = np.ascontiguousarray(
            shard.reshape(RPC, 4, 128).transpose(1, 0, 2)
        )
        in_maps.append({"x": shard, "xc": chunked, "consts": consts})
    return in_maps


def combine(results) -> np.ndarray:
    """Sum per-core partials per batch, normalize, stack."""
    out = np.empty((B, K, E), np.float32)
    for bi in range(B):
        U = np.zeros((K, E), np.float64)
        den = np.zeros((K,), np.float64)
        for si in range(CORES_PER_B):
            r = results[bi * CORES_PER_B + si]
            U += r["u"].reshape(K, E).astype(np.float64)
            # d is [128, 2, 2]: [p, c, dup] -> k = c*128 + p, drop dup col
            den += r["d"][:, :, 0].T.reshape(K).astype(np.float64)
        out[bi] = (U / den[:, None]).astype(np.float32)
    return out


_NC_CACHE: dict[int, bass.Bass] = {}


def kernel(x: np.ndarray, W: np.ndarray, b_bias: np.ndarray) -> np.ndarray:
    # b_bias shifts every column of scores by a constant along the softmax
    # axis -> cancels in softmax; unused by construction.
    if RPC not in _NC_CACHE:
        _NC_CACHE[RPC] = build_nc(RPC)
    res = _run(_NC_CACHE[RPC], make_in_maps(np.asarray(x), np.asarray(W)))
    return combine(res.results)


# revision 38
# speedup vs baseline: 1.8243x; 1.2734x over previous
"""DifferentiableTokenSelection Trainium2 kernel (bf16 pipeline).

Math (reference):
    x: [b=2, t=64, n=1024, e=512] -> x_flat [b, m=65536, e]
    scores  = x_flat @ W.T + bias            [b, m, k=256]
    weights = softmax(scores / tau, axis=m)  (tau = 1.0)
    out     = einsum('bmk,bme->bke', weights, x_flat)   [b, 256, 512]

Key simplifications (exact, not approximations):
  * softmax over m is invariant to per-(b,k) constant shifts -> the bias
    cancels entirely; ignore b_bias.
  * scores ~ N(0,1), max |s| ~ 6 -> exp() without max-subtraction is safe
    in fp32. Single streaming pass: U[k,e] = sum_m exp(s[m,k]) x[m,e] and
    denom[k] = sum_m exp(s[m,k]) accumulate in PSUM; out = U / denom.
  * numerator and denominator use the SAME quantized weights, so weight
    quantization largely cancels in the ratio.

Inputs are cast to bf16 on the host. This halves HBM traffic (x read
twice: once natural for the pooling matmul, once via the hardware DMA
transpose engine for the scores matmul) and runs all matmuls at the PE's
full 1 cycle/row rate with hidden (FWL) weight loads — fp32 matmuls are
4x slower and fp32r fused loads serialize. PSUM accumulation stays fp32.

Sharding: batch x token-axis. core i handles batch i//4, m-rows
[16384*(i%4), 16384*(i%4+1)). Each core emits partial U and denom; the
host sums the 4 partials per batch and divides (gather/unshard step).

Per-core pipeline over 512-row blocks (4x 128-row subtiles):
  DMA x natural [128,4,512] + 4x DMA-transpose xT chunks [128e, 512m];
  per subtile: mm1 scores[m,k] (4 K-chunks, PSUM accum) -> ACT exp ->
  mm2 U[k,e] += wexp^T @ x and den[k] += wexp^T @ ones (PSUM, full-kernel
  accumulation).
"""

import numpy as np
import ml_dtypes

import concourse.bacc as bacc
import concourse.bass as bass
import concourse.tile as tile
from concourse import mybir
from concourse.bass_utils import run_bass_kernel_spmd

B, T, NTOK, E, K = 2, 64, 1024, 512, 256
M = T * NTOK                 # 65536 tokens per batch
NCORES = 8
CORES_PER_B = NCORES // B    # 4
RPC = M // CORES_PER_B       # 16384 rows per core

F32 = mybir.dt.float32
BF16 = mybir.dt.bfloat16
EXP = mybir.ActivationFunctionType.Exp
BF = ml_dtypes.bfloat16

# const layout per partition: [ ones(2) | wt(4*256) ]
C_ONES, C_WT = 0, 2
C_TOT = 2 + 4 * K


def build_nc(
    rows: int,
    subs_per_blk: int = 16,
    xin_bufs: int = 3,
    xt_bufs: int = 6,
    tsplit: int = 2,
) -> bass.Bass:
    """Emit the per-core bass program for `rows` m-rows."""
    assert rows % (128 * subs_per_blk) == 0
    nsub = rows // 128
    nblk = nsub // subs_per_blk

    nc = bacc.Bacc("TRN2", target_bir_lowering=False, debug=False)
    x_d = nc.dram_tensor("x", [rows, E], BF16, kind="ExternalInput")
    # host-pre-transposed copy: xt[ec, p, m] = x[m, 128*ec + p].
    # Loading x^T tiles is then a plain strided DMA (4KB contiguous per
    # partition) — no on-device transposes at all, so no PE transpose tax
    # and no xbar DMA-transpose serialization.
    xt_d = nc.dram_tensor("xt", [4, 128, rows], BF16, kind="ExternalInput")
    c_d = nc.dram_tensor("consts", [128, C_TOT], BF16, kind="ExternalInput")
    u_d = nc.dram_tensor("u", [2, 128, E], F32, kind="ExternalOutput")
    d_d = nc.dram_tensor("d", [128, 2, 2], F32, kind="ExternalOutput")

    with tile.TileContext(nc) as tc:
        with (
            tc.tile_pool(name="const", bufs=1) as constp,
            tc.tile_pool(name="xin", bufs=xin_bufs) as xinp,
            tc.tile_pool(name="xt", bufs=xt_bufs) as xtp,
            tc.tile_pool(name="wexp", bufs=3) as wexpp,
            tc.tile_pool(name="outs", bufs=1) as outp,
            tc.tile_pool(name="ps_sc", bufs=3, space="PSUM") as ps_sc,
            tc.tile_pool(name="ps_acc", bufs=1, space="PSUM") as ps_acc,
        ):
            consts = constp.tile([128, C_TOT], BF16)
            nc.sync.dma_start(out=consts[:], in_=c_d.ap())
            ones = consts[:, C_ONES : C_ONES + 2]

            u_ps = ps_acc.tile([128, 2, E], F32)    # 2 banks, live all kernel
            den_ps = ps_acc.tile([128, 2, 2], F32)  # 1 bank; [:, c, :] pairs

            for blk in range(nblk):
                r0 = blk * subs_per_blk * 128
                xb = xinp.tile([128, subs_per_blk, E], BF16, tag="xb")
                # natural loads ride SWDGE (gpsimd) so the HWDGE (sync)
                # stream carries ONLY xbar-transposes -> no xbar-mode
                # transition serialization between the two DMA kinds
                nc.gpsimd.dma_start(
                    out=xb[:],
                    in_=x_d.ap()[r0 : r0 + subs_per_blk * 128, :].rearrange(
                        "(j p) e -> p j e", p=128
                    ),
                )
                # x^T chunks: plain DMA from the host-transposed copy
                xtb = xtp.tile([128, 4, subs_per_blk * 128], BF16, tag="xtb")
                part = subs_per_blk * 128 // tsplit
                for h in range(tsplit):
                    nc.sync.dma_start(
                        out=xtb[:, :, h * part : (h + 1) * part],
                        in_=xt_d.ap()[
                            :, :, r0 + h * part : r0 + (h + 1) * part
                        ].rearrange("c p m -> p c m"),
                    )
                for j in range(subs_per_blk):
                    it = blk * subs_per_blk + j
                    first, last = it == 0, it == nsub - 1
                    # -- mm1: scores[m,k] = sum_e x[m,e] WT[e,k]
                    sc_ps = ps_sc.tile([128, K], F32, tag="scps")
                    for ec in range(4):
                        nc.tensor.matmul(
                            sc_ps[:],
                            xtb[:, ec, j * 128 : (j + 1) * 128],
                            consts[:, C_WT + ec * K : C_WT + (ec + 1) * K],
                            start=(ec == 0),
                            stop=(ec == 3),
                        )
                    # -- exp (tau=1, bias cancels)
                    wexp = wexpp.tile([128, K], BF16, tag="wexp")
                    nc.scalar.activation(wexp[:], sc_ps[:], EXP)
                    # -- mm2: U[k,e] += wexp^T @ x ; den[k] += wexp^T @ 1
                    for c in range(2):
                        wchunk = wexp[:, c * 128 : (c + 1) * 128]
                        nc.tensor.matmul(
                            u_ps[:, c, :],
                            wchunk,
                            xb[:, j, :],
                            start=first,
                            stop=last,
                        )
                        # start=True clears has_written for the WHOLE bank;
                        # both den chunks share one bank, so only chunk 0
                        # issues it (chunk 1 then overwrites where bits are
                        # unset, which is equivalent).
                        nc.tensor.matmul(
                            den_ps[:, c, :],
                            wchunk,
                            ones,
                            start=first and c == 0,
                            stop=last,
                        )

            u_sb = outp.tile([128, 2, E], F32)
            den_sb = outp.tile([128, 2, 2], F32)
            nc.vector.tensor_copy(u_sb[:], u_ps[:])
            nc.vector.tensor_copy(den_sb[:], den_ps[:])
            nc.sync.dma_start(
                out=u_d.ap().rearrange("c p e -> p c e"), in_=u_sb[:]
            )
            nc.sync.dma_start(out=d_d.ap(), in_=den_sb[:])
    nc.compile()
    return nc


def _run(nc: bass.Bass, in_maps, **kw):
    return run_bass_kernel_spmd(nc, in_maps, list(range(len(in_maps))), **kw)


def make_consts(W: np.ndarray) -> np.ndarray:
    """[ones | W.T as [c p] k chunks] per partition, bf16."""
    consts = np.zeros((128, C_TOT), BF)
    consts[:, C_ONES : C_ONES + 2] = BF(1.0)
    wt = np.ascontiguousarray(W.T, np.float32).astype(BF)  # [E, K]
    for c in range(4):
        consts[:, C_WT + c * K : C_WT + (c + 1) * K] = wt[
            c * 128 : (c + 1) * 128, :
        ]
    return consts


def make_in_maps(x: np.ndarray, W: np.ndarray):
    xf = np.asarray(x, np.float32).reshape(B, M, E).astype(BF)
    consts = make_consts(W)
    in_maps = []
    for i in range(NCORES):
        bi, si = divmod(i, CORES_PER_B)
        shard = np.ascontiguousarray(xf[bi, si * RPC : (si + 1) * RPC])
        # xt[ec, p, m] = shard[m, 128*ec + p]
        xt = np.ascontiguousarray(
            shard.reshape(RPC, 4, 128).transpose(1, 2, 0)
        )
        in_maps.append({"x": shard, "xt": xt, "consts": consts})
    return in_maps


def combine(results) -> np.ndarray:
    """Sum per-core partials per batch, normalize, stack."""
    out = np.empty((B, K, E), np.float32)
    for bi in range(B):
        U = np.zeros((K, E), np.float64)
        den = np.zeros((K,), np.float64)
        for si in range(CORES_PER_B):
            r = results[bi * CORES_PER_B + si]
            U += r["u"].reshape(K, E).astype(np.float64)
            # d is [128, 2, 2]: [p, c, dup] -> k = c*128 + p, drop dup col
            den += r["d"][:, :, 0].T.reshape(K).astype(np.float64)
        out[bi] = (U / den[:, None]).astype(np.float32)
    return out


_NC_CACHE: dict[int, bass.Bass] = {}


def kernel(x: np.ndarray, W: np.ndarray, b_bias: np.ndarray) -> np.ndarray:
    # b_bias shifts every column of scores by a constant along the softmax
    # axis -> cancels in softmax; unused by construction.
    if RPC not in _NC_CACHE:
        _NC_CACHE[RPC] = build_nc(RPC)
    res = _run(_NC_CACHE[RPC], make_in_maps(np.asarray(x), np.asarray(W)))
    return combine(res.results)
